# revision 1
# baseline (speedup 1.0000x reference)
"""Swin-style windowed local self-attention (LN -> QKV -> 7x7 window MHA
with relative position bias -> proj) on 8 Trainium2 NeuronCores.

Sharding: pure data parallel over B*T (24 images -> 3 per core).

Device-side design (per core: 9408 tokens = 192 windows = 96 window-pairs):
  - host folds ln_g + attention scale into the QKV weights, ships bf16
    weights; window reordering of x / output happens on host (numpy).
  - LN on [98,384] token tiles (bn_stats/bn_aggr), normalized output cast
    to bf16, padded to 112 rows for the DMA-xbar transpose.
  - x^T (feature-major) built with DMA transpose; QKV q/k computed
    feature-major (weights stationary, token chunks N<=512), v computed
    token-major per window with PSUM column tiling (w0 rows 0-48,
    w1 rows 64-112 -> concurrent PE sub-tiles).
  - scores are computed transposed (S^T = [k,q]) so that after bias+exp
    the [k,q] tile IS the lhsT of the P@V matmul -- no transpose of the
    softmax matrix is ever needed.  Relative-position bias is accumulated
    into the score PSUM by an extra PE matmul against a duplicated
    identity.  exp runs on the scalar engine straight out of PSUM.
  - softmax normalization: V is augmented with a ones column, so the P@V
    matmul also produces the row sums; a reciprocal + broadcast-multiply
    at PSUM evacuation normalizes (per-partition = per-query-token).
  - attention output is transposed back to feature-major by DMA transpose
    and hits the proj matmul (token-major out), bias-added and stored.
"""

import sys

if "/opt/trn_rl_repo" not in sys.path:
    sys.path.insert(0, "/opt/trn_rl_repo")

import numpy as np
import ml_dtypes

import concourse.bacc as bacc
import concourse.bass as bass
import concourse.tile as tile
import concourse.mybir as mybir
from concourse.bass_utils import run_bass_kernel_spmd

F32 = mybir.dt.float32
BF16 = mybir.dt.bfloat16

N_CORES = 8
B, T, H, W, D = 4, 6, 56, 56, 384
WSZ = 7
NH = 12
HD = D // NH            # 32
N = WSZ * WSZ           # 49 tokens / window
NW_IMG = (H // WSZ) * (W // WSZ)   # 64 windows / image
IMGS_CORE = (B * T) // N_CORES     # 3
TOK_CORE = IMGS_CORE * H * W       # 9408
NPAIR = TOK_CORE // (2 * N)        # 96 window pairs / core
EPS = 1e-5

# group = contiguous run of window pairs whose x^T / q / k stay in SBUF
N_GROUPS = 4
PAIRS_G = NPAIR // N_GROUPS        # 24
TOK_G = PAIRS_G * 2 * N            # 2352
QK_CHUNK = 512


def _rel_index(w):
    coords = np.stack(np.meshgrid(np.arange(w), np.arange(w), indexing="ij")).reshape(2, -1)
    rel = coords[:, :, None] - coords[:, None, :]
    return (rel[0] + w - 1) * (2 * w - 1) + (rel[1] + w - 1)


def build_program(n_groups=N_GROUPS, debug_dump=False, repeat=1):
    nc = bacc.Bacc("TRN2", target_bir_lowering=False, debug=False, num_devices=N_CORES)

    x_d = nc.dram_tensor("x", [TOK_CORE, D], F32, kind="ExternalInput")
    qkw_d = nc.dram_tensor("qkw", [D, 2 * D], BF16, kind="ExternalInput")
    vw_d = nc.dram_tensor("vw", [D, D], BF16, kind="ExternalInput")
    pw_d = nc.dram_tensor("pw", [D, D], BF16, kind="ExternalInput")
    cqk_d = nc.dram_tensor("cqk", [2 * D], F32, kind="ExternalInput")
    cv_d = nc.dram_tensor("cv", [D], F32, kind="ExternalInput")
    pb_d = nc.dram_tensor("pb", [D], F32, kind="ExternalInput")
    bmm_d = nc.dram_tensor("bmm", [128, NH * N], BF16, kind="ExternalInput")
    idup_d = nc.dram_tensor("idup", [128, 64], BF16, kind="ExternalInput")
    y_d = nc.dram_tensor("y", [TOK_CORE, D], F32, kind="ExternalOutput")
    if debug_dump:
        xT_o = nc.dram_tensor("xT_o", [3, 128, PAIRS_G * 112], BF16, kind="ExternalOutput")
        qk_o = nc.dram_tensor("qk_o", [6, 128, TOK_G], BF16, kind="ExternalOutput")
        av_o = nc.dram_tensor("av_o", [PAIRS_G, 128, NH * (HD + 1)], BF16, kind="ExternalOutput")
        pt_o = nc.dram_tensor("pt_o", [PAIRS_G, 128, NH * N], BF16, kind="ExternalOutput")

    from contextlib import ExitStack
    with tile.TileContext(nc) as tc, ExitStack() as ctx:
        const = ctx.enter_context(tc.tile_pool(name="const", bufs=1))
        grp = ctx.enter_context(tc.tile_pool(name="grp", bufs=2))
        work = ctx.enter_context(tc.tile_pool(name="work", bufs=3))
        small = ctx.enter_context(tc.tile_pool(name="small", bufs=4))
        ps_qk = ctx.enter_context(tc.tile_pool(name="ps_qk", bufs=2, space="PSUM"))
        ps_v = ctx.enter_context(tc.tile_pool(name="ps_v", bufs=1, space="PSUM"))
        ps_s = ctx.enter_context(tc.tile_pool(name="ps_s", bufs=2, space="PSUM"))
        ps_o = ctx.enter_context(tc.tile_pool(name="ps_o", bufs=2, space="PSUM"))
        ps_p = ctx.enter_context(tc.tile_pool(name="ps_p", bufs=1, space="PSUM"))

        # ---- resident constants -------------------------------------------------
        qkw_sb = [const.tile([128, 2 * D], BF16, name=f"qkw{k}", tag=f"qkw{k}") for k in range(3)]
        vw_sb = [const.tile([128, D], BF16, name=f"vw{k}", tag=f"vw{k}") for k in range(3)]
        pw_sb = [const.tile([128, D], BF16, name=f"pw{k}", tag=f"pw{k}") for k in range(3)]
        for k in range(3):
            nc.sync.dma_start(out=qkw_sb[k][:], in_=qkw_d[128 * k:128 * (k + 1), :])
            nc.sync.dma_start(out=vw_sb[k][:], in_=vw_d[128 * k:128 * (k + 1), :])
            nc.sync.dma_start(out=pw_sb[k][:], in_=pw_d[128 * k:128 * (k + 1), :])
        cqk_sb = [const.tile([128, 1], F32, name=f"cqk{m}", tag=f"cqk{m}") for m in range(6)]
        for m in range(6):
            nc.sync.dma_start(out=cqk_sb[m][:], in_=cqk_d[128 * m:128 * (m + 1)])
        def bcast128(dram_ap):
            return bass.AP(tensor=dram_ap.tensor, offset=dram_ap.offset,
                           ap=[[0, 128], *dram_ap.ap])

        cv_sb = const.tile([128, D], F32, name="cv", tag="cv")
        nc.sync.dma_start(out=cv_sb[:], in_=bcast128(cv_d[:]))
        pb_sb = const.tile([128, D], F32, name="pb", tag="pb")
        nc.sync.dma_start(out=pb_sb[:], in_=bcast128(pb_d[:]))
        bmm_sb = const.tile([128, NH * N], BF16, name="bmm", tag="bmm")
        nc.sync.dma_start(out=bmm_sb[:], in_=bmm_d[:])
        idup_sb = const.tile([128, 64], BF16, name="idup", tag="idup")
        nc.sync.dma_start(out=idup_sb[:], in_=idup_d[:])
        eps_sb = const.tile([128, 1], F32, name="eps", tag="eps")
        nc.vector.memset(eps_sb[:], EPS)

        # persistent rotating tiles whose pad regions are initialized once
        NROT = 3
        xn_rot = [const.tile([128, D], BF16, name=f"xn{i}", tag=f"xn{i}") for i in range(NROT)]
        for t in xn_rot:
            # zero the transpose pad rows 98-111 once (96-aligned start; rows
            # 96-97 are rewritten by every normalize before any transpose reads)
            nc.gpsimd.memset(t[96:112, :], 0.0)
        av_rot = [const.tile([128, NH, HD + 1], BF16, name=f"av{i}", tag=f"av{i}") for i in range(NROT)]
        for t in av_rot:
            nc.gpsimd.memset(t[:, :, HD:HD + 1], 1.0)

        rep_ctx = tc.For_i(0, repeat, 1) if repeat > 1 else None
        if rep_ctx is not None:
            rep_ctx.__enter__()
        for g in range(n_groups):
            T0 = g * TOK_G
            xT = [grp.tile([128, PAIRS_G * 112], BF16, name=f"xT{k}", tag=f"xT{k}") for k in range(3)]
            qk = [grp.tile([128, TOK_G], BF16, name=f"qk{m}", tag=f"qk{m}") for m in range(6)]

            # ---- phase A: LN + transpose ---------------------------------------
            for p in range(PAIRS_G):
                r0 = T0 + 98 * p
                x_t = work.tile([128, D], F32, name="x", tag="x")
                nc.sync.dma_start(out=x_t[0:98, :], in_=x_d[r0:r0 + 98, :])
                stats = small.tile([128, 6], F32, name="stats", tag="stats")
                nc.vector.bn_stats(out=stats[0:98, :], in_=x_t[0:98, :])
                mv = small.tile([128, 2], F32, name="mv", tag="mv")
                nc.vector.bn_aggr(out=mv[0:98, :], in_=stats[0:98, :])
                nc.scalar.activation(
                    out=mv[0:98, 1:2], in_=mv[0:98, 1:2],
                    func=mybir.ActivationFunctionType.Sqrt,
                    bias=eps_sb[0:98, :], scale=1.0,
                )
                nc.vector.reciprocal(out=mv[0:98, 1:2], in_=mv[0:98, 1:2])
                xn = xn_rot[p % NROT]
                nc.vector.tensor_scalar(
                    out=xn[0:98, :], in0=x_t[0:98, :],
                    scalar1=mv[0:98, 0:1], scalar2=mv[0:98, 1:2],
                    op0=mybir.AluOpType.subtract, op1=mybir.AluOpType.mult,
                )
                for k in range(3):
                    # xbar transpose: out column offset must be 16-aligned,
                    # hence the 112-wide per-pair slots
                    nc.sync.dma_start(
                        out=xT[k][:, 112 * p:112 * p + 112],
                        in_=xn[0:112, 128 * k:128 * (k + 1)],
                        transpose=True,
                    )

            # ---- phase B: q/k projections (feature-major) ----------------------
            # rhs is a strided view skipping the 14 pad cols of each 112-slot
            PCH = 5
            for pc in range(0, PAIRS_G, PCH):
                np_ = min(PCH, PAIRS_G - pc)
                w = 98 * np_
                for m in range(6):
                    pq = ps_qk.tile([128, QK_CHUNK], F32, name="pqk", tag="pqk")
                    for k in range(3):
                        xTv = xT[k][:].rearrange("f (p c) -> f p c", c=112)
                        nc.tensor.matmul(
                            pq[:, 0:w],
                            lhsT=qkw_sb[k][:, 128 * m:128 * (m + 1)],
                            rhs=xTv[:, pc:pc + np_, 0:98],
                            start=(k == 0), stop=(k == 2),
                        )
                    nc.vector.tensor_scalar(
                        out=qk[m][:, 98 * pc:98 * pc + w], in0=pq[:, 0:w],
                        scalar1=cqk_sb[m][:], scalar2=None,
                        op0=mybir.AluOpType.add,
                    )

            if debug_dump and g == 0:
                for k in range(3):
                    nc.sync.dma_start(out=xT_o[k], in_=xT[k][:])
                for m in range(6):
                    nc.sync.dma_start(out=qk_o[m], in_=qk[m][:])

            # ---- phase C: per window pair --------------------------------------
            for p in range(PAIRS_G):
                # v projection: window w01 -> psum rows 64*w01..+49 (col tiling)
                pv = ps_v.tile([128, D], F32, name="pv", tag="pv")
                for w01 in range(2):
                    c0 = 112 * p + 49 * w01
                    for k in range(3):
                        nc.tensor.matmul(
                            pv[64 * w01:64 * w01 + 49, :],
                            lhsT=xT[k][:, c0:c0 + 49],
                            rhs=vw_sb[k][:],
                            start=(k == 0), stop=(k == 2),
                        )
                av = av_rot[p % NROT]
                nc.vector.tensor_tensor(
                    out=av[0:113, :, 0:HD],
                    in0=pv[0:113, :].rearrange("p (h d) -> p h d", d=HD),
                    in1=cv_sb[0:113, :].rearrange("p (h d) -> p h d", d=HD),
                    op=mybir.AluOpType.add,
                )

                if debug_dump and g == 0:
                    nc.sync.dma_start(out=av_o[p], in_=av[:].rearrange("p h d -> p (h d)"))

                # scores S^T[k,q] per (window, head) + bias matmul + exp
                p_t = work.tile([128, NH, N], BF16, name="pt", tag="pt")
                for quad in range(3):
                    ps = ps_s.tile([128, 4, N], F32, name="ps", tag="ps")
                    for j in range(4):
                        h = 4 * quad + j
                        qt = qk[h // 4]
                        kt = qk[3 + h // 4]
                        hb = 32 * (h % 4)
                        for w01 in range(2):
                            c0 = 98 * p + 49 * w01
                            ob = 64 * w01
                            nc.tensor.matmul(
                                ps[ob:ob + 49, j, :],
                                lhsT=kt[hb:hb + 32, c0:c0 + 49],
                                rhs=qt[hb:hb + 32, c0:c0 + 49],
                                start=True, stop=False,
                                tile_position=(hb, ob),
                            )
                            nc.tensor.matmul(
                                ps[ob:ob + 49, j, :],
                                lhsT=bmm_sb[ob:ob + 49, N * h:N * (h + 1)],
                                rhs=idup_sb[ob:ob + 49, 0:49],
                                start=False, stop=True,
                            )
                    nc.scalar.activation(
                        out=p_t[0:113, 4 * quad:4 * quad + 4, :],
                        in_=ps[0:113, :, :],
                        func=mybir.ActivationFunctionType.Exp,
                    )

                if debug_dump and g == 0:
                    nc.sync.dma_start(out=pt_o[p], in_=p_t[:].rearrange("p h n -> p (h n)"))

                # P @ [V | 1]  (lhsT is p_t directly -- already [k, q])
                po = ps_o.tile([128, NH, HD + 1], F32, name="po", tag="po")
                for h in range(NH):
                    for w01 in range(2):
                        ob = 64 * w01
                        nc.tensor.matmul(
                            po[ob:ob + 49, h, :],
                            lhsT=p_t[ob:ob + 49, h, :],
                            rhs=av[ob:ob + 49, h, :],
                            start=True, stop=True,
                        )
                rec = small.tile([128, NH], F32, name="rec", tag="rec")
                nc.vector.reciprocal(out=rec[0:113, :], in_=po[0:113, :, HD])
                at_sb = work.tile([128, D], BF16, name="at", tag="at")
                rec_sl = rec[0:113, :]
                rec_b = bass.AP(
                    tensor=rec_sl.tensor,
                    offset=rec_sl.offset,
                    ap=[*rec_sl.ap, [0, HD]],
                )
                nc.vector.tensor_tensor(
                    out=at_sb[0:113, :].rearrange("p (h d) -> p h d", d=HD),
                    in0=po[0:113, :, 0:HD],
                    in1=rec_b,
                    op=mybir.AluOpType.mult,
                )

                # transpose attention out to feature-major, proj, bias, store
                at_T = work.tile([128, 3 * 128], BF16, name="atT", tag="atT")
                for k in range(3):
                    nc.sync.dma_start(
                        out=at_T[:, 128 * k:128 * (k + 1)],
                        in_=at_sb[0:128, 128 * k:128 * (k + 1)],
                        transpose=True,
                    )
                pp = ps_p.tile([128, D], F32, name="pp", tag="pp")
                for w01 in range(2):
                    ob = 64 * w01
                    for k in range(3):
                        nc.tensor.matmul(
                            pp[ob:ob + 49, :],
                            lhsT=at_T[:, 128 * k + ob:128 * k + ob + 49],
                            rhs=pw_sb[k][:],
                            start=(k == 0), stop=(k == 2),
                        )
                y_sb = work.tile([128, D], F32, name="y", tag="y")
                nc.vector.tensor_tensor(
                    out=y_sb[0:113, :], in0=pp[0:113, :], in1=pb_sb[0:113, :],
                    op=mybir.AluOpType.add,
                )
                r0 = T0 + 98 * p
                nc.sync.dma_start(out=y_d[r0:r0 + 49, :], in_=y_sb[0:49, :])
                nc.sync.dma_start(out=y_d[r0 + 49:r0 + 98, :], in_=y_sb[64:113, :])
        if rep_ctx is not None:
            rep_ctx.__exit__(None, None, None)

    nc.compile()
    return nc


_NC_CACHE = {}


def _get_program():
    if "nc" not in _NC_CACHE:
        _NC_CACHE["nc"] = build_program()
    return _NC_CACHE["nc"]


def _window_order(xf):
    # [BT, H, W, D] -> [BT*nW*N, D] in window-raster order
    BT = xf.shape[0]
    x6 = xf.reshape(BT, H // WSZ, WSZ, W // WSZ, WSZ, D)
    return np.ascontiguousarray(x6.transpose(0, 1, 3, 2, 4, 5)).reshape(-1, D)


def _window_unorder(yw):
    BT = B * T
    y6 = yw.reshape(BT, H // WSZ, W // WSZ, WSZ, WSZ, D)
    return np.ascontiguousarray(y6.transpose(0, 1, 3, 2, 4, 5)).reshape(BT, H, W, D)


def prepare_inputs(x, ln_g, ln_b, qkv_w, qkv_b, proj_w, proj_b, rel_bias_table):
    x = np.asarray(x, np.float32)
    ln_g = np.asarray(ln_g, np.float32)
    ln_b = np.asarray(ln_b, np.float32)
    qkv_w = np.asarray(qkv_w, np.float32)
    qkv_b = np.asarray(qkv_b, np.float32)
    proj_w = np.asarray(proj_w, np.float32)
    proj_b = np.asarray(proj_b, np.float32)
    rel_bias_table = np.asarray(rel_bias_table, np.float32)

    scale = HD ** -0.5
    wq = qkv_w[:, :D] * ln_g[:, None] * scale
    wk = qkv_w[:, D:2 * D] * ln_g[:, None]
    wv = qkv_w[:, 2 * D:] * ln_g[:, None]
    cq = (ln_b @ qkv_w[:, :D] + qkv_b[:D]) * scale
    ck = ln_b @ qkv_w[:, D:2 * D] + qkv_b[D:2 * D]
    cv = ln_b @ qkv_w[:, 2 * D:] + qkv_b[2 * D:]

    qkw = np.concatenate([wq, wk], axis=1).astype(ml_dtypes.bfloat16)
    cqk = np.concatenate([cq, ck]).astype(np.float32)

    idx = _rel_index(WSZ)
    bias = rel_bias_table[idx.reshape(-1)].reshape(N, N, NH)  # [q, k, h]
    bmm = np.zeros((128, NH * N), np.float32)
    for h in range(NH):
        bmm[0:49, N * h:N * (h + 1)] = bias[:, :, h]
        bmm[64:113, N * h:N * (h + 1)] = bias[:, :, h]

    idup = np.zeros((128, 64), np.float32)
    idup[0:49, 0:49] = np.eye(49)
    idup[64:113, 0:49] = np.eye(49)

    xw = _window_order(x.reshape(B * T, H, W, D))

    common = {
        "qkw": qkw,
        "vw": wv.astype(ml_dtypes.bfloat16),
        "pw": proj_w.astype(ml_dtypes.bfloat16),
        "cqk": cqk,
        "cv": cv.astype(np.float32),
        "pb": proj_b.astype(np.float32),
        "bmm": bmm.astype(ml_dtypes.bfloat16),
        "idup": idup.astype(ml_dtypes.bfloat16),
    }
    in_maps = []
    for c in range(N_CORES):
        m = dict(common)
        m["x"] = np.ascontiguousarray(xw[TOK_CORE * c:TOK_CORE * (c + 1)])
        in_maps.append(m)
    return in_maps


def kernel(x, ln_g, ln_b, qkv_w, qkv_b, proj_w, proj_b, rel_bias_table):
    nc = _get_program()
    in_maps = prepare_inputs(x, ln_g, ln_b, qkv_w, qkv_b, proj_w, proj_b, rel_bias_table)
    res = run_bass_kernel_spmd(nc, in_maps, core_ids=list(range(N_CORES)))
    yw = np.concatenate([res.results[c]["y"] for c in range(N_CORES)], axis=0)
    out = _window_unorder(yw).reshape(B, T, H, W, D)
    return out.astype(np.float32)



# revision 10
# speedup vs baseline: 3666.4477x; 3666.4477x over previous
"""Swin-style windowed local self-attention (LN -> QKV -> 7x7 window MHA
with relative position bias -> proj) on 8 Trainium2 NeuronCores.

Sharding: pure data parallel over B*T (24 images -> 3 per core).

v2 design (per core: 9408 tokens = 96 window pairs, 8 groups of 12):
  - window PAIRS are stacked: 98 tokens (2 windows) live on partitions
    0-97 of every working tile; cross-window attention terms are killed
    by multiplying exp(scores) with a constant exp(bias) mask that is 0
    on the off-diagonal window blocks (and exp(rel-pos bias) on the
    diagonal).  This halves the matmul count of the attention stage and
    removes the bias matmuls entirely.
  - all transposes run on the tensor engine (transpose-mode matmul into
    PSUM + one evacuation copy) -- zero DMA transposes.
  - LayerNorm rsqrt is computed as exp(-0.5*ln(var+eps)) so the scalar
    engine only ever uses the natural_log_exp_and_others activation
    table (ln/exp/identity): no activation-table reloads.
  - x is loaded with 2 big strided DMAs per group; y is staged in a
    group SBUF tile and stored with 2 big DMAs per group.  ~50 DMA
    instructions per core total (baseline: ~880).
  - softmax normalization: V is augmented with a ones column, so P@V
    also yields row sums; reciprocal + per-partition broadcast multiply
    normalizes at PSUM evacuation (tokens are the partition dim there).
  - LN bias/scale, qkv bias, v bias and proj bias are folded on the
    host; cv/pb ride the PE as rank-1 matmuls (ones-row lhsT).
"""

import sys

if "/opt/trn_rl_repo" not in sys.path:
    sys.path.insert(0, "/opt/trn_rl_repo")

import numpy as np
import ml_dtypes

import concourse.bacc as bacc
import concourse.bass as bass
import concourse.tile as tile
import concourse.mybir as mybir
from concourse.bass_utils import run_bass_kernel_spmd

F32 = mybir.dt.float32
BF16 = mybir.dt.bfloat16
AF = mybir.ActivationFunctionType
OP = mybir.AluOpType

N_CORES = 8
B, T, H, W, D = 4, 6, 56, 56, 384
WSZ = 7
NH = 12
HD = D // NH            # 32
N = WSZ * WSZ           # 49 tokens / window
NW_IMG = (H // WSZ) * (W // WSZ)   # 64 windows / image
IMGS_CORE = (B * T) // N_CORES     # 3
TOK_CORE = IMGS_CORE * H * W       # 9408
NPAIR = TOK_CORE // (2 * N)        # 96 window pairs / core
NP2 = 2 * N                        # 98 tokens / pair
EPS = 1e-5

N_GROUPS = 8
PAIRS_G = NPAIR // N_GROUPS        # 12
TOK_G = PAIRS_G * NP2              # 1176
PCH = 4                            # pairs per q/k projection chunk


def _rel_index(w):
    coords = np.stack(np.meshgrid(np.arange(w), np.arange(w), indexing="ij")).reshape(2, -1)
    rel = coords[:, :, None] - coords[:, None, :]
    return (rel[0] + w - 1) * (2 * w - 1) + (rel[1] + w - 1)


def build_program(repeat=1, use_gpsimd=True, use_rank1=True, stage=6, sub4=''):
    nc = bacc.Bacc("TRN2", target_bir_lowering=False, debug=False, num_devices=N_CORES)

    x_d = nc.dram_tensor("x", [TOK_CORE, D], F32, kind="ExternalInput")
    qkw_d = nc.dram_tensor("qkw", [D, 2 * D], BF16, kind="ExternalInput")
    vw_d = nc.dram_tensor("vw", [D, D], BF16, kind="ExternalInput")
    pw_d = nc.dram_tensor("pw", [D, D], BF16, kind="ExternalInput")
    cqk_d = nc.dram_tensor("cqk", [2 * D], F32, kind="ExternalInput")
    cvpb_d = nc.dram_tensor("cvpb", [2 * D], BF16, kind="ExternalInput")
    bmm_d = nc.dram_tensor("bmm", [NP2, NH * NP2], BF16, kind="ExternalInput")
    ident_d = nc.dram_tensor("ident", [NP2, NP2], BF16, kind="ExternalInput")
    y_d = nc.dram_tensor("y", [TOK_CORE, D], F32, kind="ExternalOutput")

    from contextlib import ExitStack
    with tile.TileContext(nc) as tc, ExitStack() as ctx:
        const = ctx.enter_context(tc.tile_pool(name="const", bufs=1))
        grp = ctx.enter_context(tc.tile_pool(name="grp", bufs=2))
        work = ctx.enter_context(tc.tile_pool(name="work", bufs=3))
        small = ctx.enter_context(tc.tile_pool(name="small", bufs=4))
        ps_tr = ctx.enter_context(tc.tile_pool(name="ps_tr", bufs=2, space="PSUM"))
        ps_mm = ctx.enter_context(tc.tile_pool(name="ps_mm", bufs=2, space="PSUM"))
        ps_s = ctx.enter_context(tc.tile_pool(name="ps_s", bufs=2, space="PSUM"))
        ps_o = ctx.enter_context(tc.tile_pool(name="ps_o", bufs=2, space="PSUM"))

        # ---- resident constants -------------------------------------------------
        qkw_sb = [const.tile([128, 2 * D], BF16, name=f"qkw{k}", tag=f"qkw{k}") for k in range(3)]
        vw_sb = [const.tile([128, D], BF16, name=f"vw{k}", tag=f"vw{k}") for k in range(3)]
        pw_sb = [const.tile([128, D], BF16, name=f"pw{k}", tag=f"pw{k}") for k in range(3)]
        for k in range(3):
            nc.sync.dma_start(out=qkw_sb[k][:], in_=qkw_d[128 * k:128 * (k + 1), :])
            nc.sync.dma_start(out=vw_sb[k][:], in_=vw_d[128 * k:128 * (k + 1), :])
            nc.sync.dma_start(out=pw_sb[k][:], in_=pw_d[128 * k:128 * (k + 1), :])
        cqk_sb = [const.tile([128, 1], F32, name=f"cqk{m}", tag=f"cqk{m}") for m in range(6)]
        for m in range(6):
            nc.sync.dma_start(out=cqk_sb[m][:], in_=cqk_d[128 * m:128 * (m + 1)])

        def row1(dram_ap, n):
            # DRAM vector -> [1, n] AP (single partition)
            return bass.AP(tensor=dram_ap.tensor, offset=dram_ap.offset,
                           ap=[[0, 1], *dram_ap.ap])

        cvrow = const.tile([128, D], BF16, name="cvrow", tag="cvrow")
        nc.sync.dma_start(out=cvrow[0:1, :], in_=row1(cvpb_d[0:D], D))
        pbrow = const.tile([128, D], BF16, name="pbrow", tag="pbrow")
        nc.sync.dma_start(out=pbrow[0:1, :], in_=row1(cvpb_d[D:2 * D], D))
        bmm_sb = const.tile([128, NH, NP2], BF16, name="bmm", tag="bmm")
        nc.sync.dma_start(out=bmm_sb[0:NP2, :, :], in_=bmm_d[:, :])
        ident_sb = const.tile([128, NP2], BF16, name="ident", tag="ident")
        nc.sync.dma_start(out=ident_sb[0:NP2, :], in_=ident_d[:, :])
        eps_sb = const.tile([128, 1], F32, name="eps", tag="eps")
        nc.vector.memset(eps_sb[:], EPS)
        ones1 = const.tile([128, NP2], BF16, name="ones1", tag="ones1")
        nc.gpsimd.memset(ones1[0:1, :], 1.0)

        NROT = 3
        av_rot = [const.tile([128, NH, HD + 1], BF16, name=f"av{i}", tag=f"av{i}") for i in range(NROT)]
        for t in av_rot:
            nc.gpsimd.memset(t[:, :, HD:HD + 1], 1.0)

        rep_ctx = tc.For_i(0, repeat, 1) if repeat > 1 else None
        if rep_ctx is not None:
            rep_ctx.__enter__()
        for g in range(N_GROUPS):
            R0 = g * TOK_G
            x_t = grp.tile([128, PAIRS_G, D], F32, name="x_t", tag="x_t")
            xT = grp.tile([128, 3, PAIRS_G, NP2], BF16, name="xT", tag="xT")
            qk = [grp.tile([128, TOK_G], BF16, name=f"qk{m}", tag=f"qk{m}") for m in range(6)]
            yg = grp.tile([128, PAIRS_G, D], F32, name="yg", tag="yg")

            hp = PAIRS_G // 2
            for hf in range(2):
                src = x_d[R0 + hf * hp * NP2: R0 + (hf + 1) * hp * NP2, :]
                nc.sync.dma_start(
                    out=x_t[0:NP2, hf * hp:(hf + 1) * hp, :],
                    in_=src.rearrange("(p t) d -> t p d", t=NP2),
                )

            # ---- phase A: LN + transpose (PE) ----------------------------------
            mv6 = small.tile([128, PAIRS_G, 6], F32, name="mv6", tag="mv6")
            mva = small.tile([128, PAIRS_G, 2], F32, name="mva", tag="mva")
            for p in range(PAIRS_G):
                nc.vector.bn_stats(out=mv6[0:NP2, p, :], in_=x_t[0:NP2, p, :])
                nc.vector.bn_aggr(out=mva[0:NP2, p, :], in_=mv6[0:NP2, p, :])
            lnv = small.tile([128, PAIRS_G], F32, name="lnv", tag="lnv")
            nc.scalar.activation(
                out=lnv[0:NP2, :], in_=mva[0:NP2, :, 1], func=AF.Ln,
                bias=eps_sb[0:NP2, :], scale=1.0,
            )
            rs = small.tile([128, PAIRS_G], F32, name="rs", tag="rs")
            nc.scalar.activation(
                out=rs[0:NP2, :], in_=lnv[0:NP2, :], func=AF.Exp, scale=-0.5,
            )
            for p in range(PAIRS_G):
                xn = work.tile([128, D], BF16, name="xn", tag="xn")
                eng_ln = nc.gpsimd if use_gpsimd else nc.vector
                eng_ln.tensor_scalar(
                    xn[0:NP2, :], x_t[0:NP2, p, :],
                    mva[0:NP2, p, 0:1], rs[0:NP2, p:p + 1],
                    op0=OP.subtract, op1=OP.mult,
                )
                pst_full = ps_tr.tile([128, 1024], BF16, name="ptr", tag="ptr")
                pst = pst_full[:, 0:3 * NP2].rearrange("p (k t) -> p k t", t=NP2)
                for k in range(3):
                    nc.tensor.transpose(
                        pst[:, k, :], xn[0:NP2, 128 * k:128 * (k + 1)],
                        ident_sb[0:NP2, :],
                    )
                nc.vector.tensor_scalar(
                    xT[:, :, p, :], pst[:, :, :], 0.0, None, op0=OP.add,
                )

            # ---- phase B: q/k projections (feature-major) ----------------------
            for c in range(PAIRS_G // PCH if stage >= 2 else 0):
                for m in range(6):
                    pq_full = ps_mm.tile([128, 512], F32, name="pmm", tag="pmm")
                    pq = pq_full[:, 0:PCH * NP2]
                    for k in range(3):
                        nc.tensor.matmul(
                            pq[:, :],
                            lhsT=qkw_sb[k][:, 128 * m:128 * (m + 1)],
                            rhs=xT[:, k, PCH * c:PCH * (c + 1), :],
                            start=(k == 0), stop=(k == 2),
                        )
                    nc.scalar.activation(
                        out=qk[m][:, PCH * NP2 * c:PCH * NP2 * (c + 1)],
                        in_=pq[:, :], func=AF.Identity,
                        bias=cqk_sb[m][:], scale=1.0,
                    )

            # ---- phase C: per window pair --------------------------------------
            for p in range(PAIRS_G if stage >= 3 else 0):
                # v projection (token-major, both windows stacked) + cv
                pv_full = ps_mm.tile([128, 512], F32, name="pmm", tag="pmm")
                pv = pv_full
                for k in range(3):
                    nc.tensor.matmul(
                        pv[0:NP2, 0:D], lhsT=xT[:, k, p, :], rhs=vw_sb[k][:],
                        start=(k == 0), stop=(k == 2 and not use_rank1),
                    )
                if use_rank1:
                    nc.tensor.matmul(
                        pv[0:NP2, 0:D], lhsT=ones1[0:1, :], rhs=cvrow[0:1, :],
                        start=False, stop=True,
                    )
                av = av_rot[p % NROT]
                nc.vector.tensor_scalar(
                    av[0:NP2, :, 0:HD],
                    pv[0:NP2, 0:D].rearrange("p (h d) -> p h d", d=HD),
                    0.0, None, op0=OP.add,
                )

                # scores S^T[k, q] (stacked pair); rel-pos bias + cross-window
                # mask ride a closing matmul (bmm^T via identity) per head, then exp
                p_t = work.tile([128, NH, NP2], BF16, name="pt", tag="pt")
                for quad in range(3):
                    ps_full = ps_s.tile([128, 512], F32, name="ps", tag="ps")
                    ps = ps_full[:, 0:4 * NP2].rearrange("p (j t) -> p j t", t=NP2)
                    qt = qk[quad]
                    kt = qk[3 + quad]
                    for j in range(4):
                        hb = 32 * j
                        h = 4 * quad + j
                        nc.tensor.matmul(
                            ps[0:NP2, j, :],
                            lhsT=kt[hb:hb + 32, NP2 * p:NP2 * (p + 1)],
                            rhs=qt[hb:hb + 32, NP2 * p:NP2 * (p + 1)],
                            start=True, stop=False,
                            tile_position=(hb, 0),
                        )
                        nc.tensor.matmul(
                            ps[0:NP2, j, :],
                            lhsT=bmm_sb[0:NP2, h, :],
                            rhs=ident_sb[0:NP2, :],
                            start=False, stop=True,
                        )
                    nc.scalar.activation(
                        out=p_t[0:NP2, 4 * quad:4 * (quad + 1), :],
                        in_=ps[0:NP2, :, :], func=AF.Exp,
                    )

                # P @ [V | 1]
                if stage < 5:
                    nc.scalar.activation(out=yg[0:NP2, p, :], in_=pv[0:NP2, 0:D], func=AF.Identity)
                    continue
                po_full = ps_o.tile([128, 512], F32, name="po", tag="po")
                po = po_full[:, 0:NH * (HD + 1)].rearrange("p (h d) -> p h d", d=HD + 1)
                for h in range(NH):
                    nc.tensor.matmul(
                        po[0:NP2, h, :], lhsT=p_t[0:NP2, h, :], rhs=av[0:NP2, h, :],
                        start=True, stop=True,
                    )
                rec = small.tile([128, NH], F32, name="rec", tag="rec")
                nc.vector.reciprocal(out=rec[0:NP2, :], in_=po[0:NP2, :, HD])
                at_sb = work.tile([128, D], BF16, name="at", tag="at")
                rec_sl = rec[0:NP2, :]
                rec_b = bass.AP(
                    tensor=rec_sl.tensor, offset=rec_sl.offset,
                    ap=[*rec_sl.ap, [0, HD]],
                )
                nc.vector.tensor_tensor(
                    out=at_sb[0:NP2, :].rearrange("p (h d) -> p h d", d=HD),
                    in0=po[0:NP2, :, 0:HD], in1=rec_b, op=OP.mult,
                )

                # transpose attention out (PE), proj (+pb), stage to yg
                if stage < 6:
                    nc.scalar.activation(out=yg[0:NP2, p, :], in_=po[0:NP2, :, 0:HD].rearrange("p h d -> p (h d)"), func=AF.Identity)
                    continue
                pat_full = ps_tr.tile([128, 1024], BF16, name="ptr", tag="ptr")
                pat = pat_full[:, 0:3 * NP2].rearrange("p (k t) -> p k t", t=NP2)
                for k in range(3):
                    nc.tensor.transpose(
                        pat[:, k, :], at_sb[0:NP2, 128 * k:128 * (k + 1)],
                        ident_sb[0:NP2, :],
                    )
                atT = work.tile([128, 3, NP2], BF16, name="atT", tag="atT")
                nc.vector.tensor_scalar(
                    atT[:, :, :], pat[:, :, :], 0.0, None, op0=OP.add,
                )
                pp_full = ps_mm.tile([128, 512], F32, name="pmm", tag="pmm")
                pp = pp_full
                if use_rank1:
                    nc.tensor.matmul(
                        pp[0:NP2, 0:D], lhsT=ones1[0:1, :], rhs=pbrow[0:1, :],
                        start=True, stop=False,
                    )
                for k in range(3):
                    nc.tensor.matmul(
                        pp[0:NP2, 0:D], lhsT=atT[:, k, :], rhs=pw_sb[k][:],
                        start=(k == 0 and not use_rank1), stop=(k == 2),
                    )
                nc.scalar.activation(
                    out=yg[0:NP2, p, :], in_=pp[0:NP2, 0:D], func=AF.Identity,
                )

            if stage < 3:
                for p in range(PAIRS_G):
                    nc.scalar.activation(out=yg[0:NP2, p, :], in_=x_t[0:NP2, p, :], func=AF.Identity)
            for hf in range(2):
                dst = y_d[R0 + hf * hp * NP2: R0 + (hf + 1) * hp * NP2, :]
                nc.sync.dma_start(
                    out=dst.rearrange("(p t) d -> t p d", t=NP2),
                    in_=yg[0:NP2, hf * hp:(hf + 1) * hp, :],
                )
        if rep_ctx is not None:
            rep_ctx.__exit__(None, None, None)

    nc.compile()
    return nc


_NC_CACHE = {}


def _get_program():
    if "nc" not in _NC_CACHE:
        _NC_CACHE["nc"] = build_program()
    return _NC_CACHE["nc"]


def _window_order(xf):
    # [BT, H, W, D] -> [BT*nW*N, D] in window-raster order
    BT = xf.shape[0]
    x6 = xf.reshape(BT, H // WSZ, WSZ, W // WSZ, WSZ, D)
    return np.ascontiguousarray(x6.transpose(0, 1, 3, 2, 4, 5)).reshape(-1, D)


def _window_unorder(yw):
    BT = B * T
    y6 = yw.reshape(BT, H // WSZ, W // WSZ, WSZ, WSZ, D)
    return np.ascontiguousarray(y6.transpose(0, 1, 3, 2, 4, 5)).reshape(BT, H, W, D)


def prepare_inputs(x, ln_g, ln_b, qkv_w, qkv_b, proj_w, proj_b, rel_bias_table):
    x = np.asarray(x, np.float32)
    ln_g = np.asarray(ln_g, np.float32)
    ln_b = np.asarray(ln_b, np.float32)
    qkv_w = np.asarray(qkv_w, np.float32)
    qkv_b = np.asarray(qkv_b, np.float32)
    proj_w = np.asarray(proj_w, np.float32)
    proj_b = np.asarray(proj_b, np.float32)
    rel_bias_table = np.asarray(rel_bias_table, np.float32)

    scale = HD ** -0.5
    wq = qkv_w[:, :D] * ln_g[:, None] * scale
    wk = qkv_w[:, D:2 * D] * ln_g[:, None]
    wv = qkv_w[:, 2 * D:] * ln_g[:, None]
    cq = (ln_b @ qkv_w[:, :D] + qkv_b[:D]) * scale
    ck = ln_b @ qkv_w[:, D:2 * D] + qkv_b[D:2 * D]
    cv = ln_b @ qkv_w[:, 2 * D:] + qkv_b[2 * D:]

    qkw = np.concatenate([wq, wk], axis=1).astype(ml_dtypes.bfloat16)
    cqk = np.concatenate([cq, ck]).astype(np.float32)
    cvpb = np.concatenate([cv, proj_b]).astype(ml_dtypes.bfloat16)

    idx = _rel_index(WSZ)
    bias = rel_bias_table[idx.reshape(-1)].reshape(N, N, NH)  # [q, k, h]
    # bmm[q, h*98+k] = bias[h][q, k] on same-window blocks, -1e30 across
    # (exp of masked scores is then exactly 0 -> correct windowed softmax)
    bmm = np.full((NP2, NH, NP2), -1e30, np.float32)
    for h in range(NH):
        blk = bias[:, :, h]  # [q, k]
        bmm[0:N, h, 0:N] = blk
        bmm[N:NP2, h, N:NP2] = blk
    bmm = bmm.reshape(NP2, NH * NP2).astype(ml_dtypes.bfloat16)

    ident = np.eye(NP2, dtype=np.float32).astype(ml_dtypes.bfloat16)

    xw = _window_order(x.reshape(B * T, H, W, D))

    common = {
        "qkw": qkw,
        "vw": wv.astype(ml_dtypes.bfloat16),
        "pw": proj_w.astype(ml_dtypes.bfloat16),
        "cqk": cqk,
        "cvpb": cvpb,
        "bmm": bmm,
        "ident": ident,
    }
    in_maps = []
    for c in range(N_CORES):
        m = dict(common)
        m["x"] = np.ascontiguousarray(xw[TOK_CORE * c:TOK_CORE * (c + 1)])
        in_maps.append(m)
    return in_maps


def kernel(x, ln_g, ln_b, qkv_w, qkv_b, proj_w, proj_b, rel_bias_table):
    nc = _get_program()
    in_maps = prepare_inputs(x, ln_g, ln_b, qkv_w, qkv_b, proj_w, proj_b, rel_bias_table)
    res = run_bass_kernel_spmd(nc, in_maps, core_ids=list(range(N_CORES)))
    yw = np.concatenate([res.results[c]["y"] for c in range(N_CORES)], axis=0)
    out = _window_unorder(yw).reshape(B, T, H, W, D)
    return out.astype(np.float32)


# revision 12
# speedup vs baseline: 22696.5387x; 6.1903x over previous
"""Swin-style windowed local self-attention (LN -> QKV -> 7x7 window MHA
with relative position bias -> proj) on 8 Trainium2 NeuronCores.

Sharding: pure data parallel over B*T (24 images -> 3 per core).

v3 design (per core: 9408 tokens = 96 window pairs, 8 groups of 12):
  - window PAIRS are stacked: 98 tokens (2 windows) live on partitions
    0-97 of every working tile; relative-position bias AND the
    cross-window mask (-1e30) are accumulated by a closing matmul
    (bmm^T via identity) per head -- this also works around a HW issue
    with >2 concurrent single-shot tile_position matmuls per PSUM bank.
  - x^T / attention-out transposes use the DMA xbar (112-wide slots for
    16-alignment), keeping the tensor engine free for matmuls.
  - LayerNorm rsqrt is computed as exp(-0.5*ln(var+eps)) so the scalar
    engine only uses the natural_log_exp_and_others activation table
    (ln/exp/identity): no activation-table reloads.
  - x is loaded with 2 big strided DMAs per group; y is staged in a
    group SBUF tile and stored with 2 big DMAs per group.
  - softmax normalization: V is augmented with a ones column, so P@V
    also yields row sums; reciprocal + per-partition broadcast multiply
    normalizes at PSUM evacuation.
  - LN bias/scale, qkv bias, v bias and proj bias are folded on the
    host; cv/pb ride the PE as rank-1 matmuls (ones-row lhsT).
"""

import sys

if "/opt/trn_rl_repo" not in sys.path:
    sys.path.insert(0, "/opt/trn_rl_repo")

import numpy as np
import ml_dtypes

import concourse.bacc as bacc
import concourse.bass as bass
import concourse.tile as tile
import concourse.mybir as mybir
from concourse.bass_utils import run_bass_kernel_spmd

F32 = mybir.dt.float32
BF16 = mybir.dt.bfloat16
AF = mybir.ActivationFunctionType
OP = mybir.AluOpType

N_CORES = 8
B, T, H, W, D = 4, 6, 56, 56, 384
WSZ = 7
NH = 12
HD = D // NH            # 32
N = WSZ * WSZ           # 49 tokens / window
IMGS_CORE = (B * T) // N_CORES     # 3
TOK_CORE = IMGS_CORE * H * W       # 9408
NPAIR = TOK_CORE // (2 * N)        # 96 window pairs / core
NP2 = 2 * N                        # 98 tokens / pair
SLOT = 112                         # 16-aligned token slot for xbar transposes
EPS = 1e-5

N_GROUPS = 8
PAIRS_G = NPAIR // N_GROUPS        # 12
TOK_G = PAIRS_G * NP2              # 1176
PCH = 4                            # pairs per q/k projection chunk


def _rel_index(w):
    coords = np.stack(np.meshgrid(np.arange(w), np.arange(w), indexing="ij")).reshape(2, -1)
    rel = coords[:, :, None] - coords[:, None, :]
    return (rel[0] + w - 1) * (2 * w - 1) + (rel[1] + w - 1)


def build_program(repeat=1):
    nc = bacc.Bacc("TRN2", target_bir_lowering=False, debug=False, num_devices=N_CORES)

    x_d = nc.dram_tensor("x", [TOK_CORE, D], F32, kind="ExternalInput")
    qkw_d = nc.dram_tensor("qkw", [D, 2 * D], BF16, kind="ExternalInput")
    vw_d = nc.dram_tensor("vw", [D, D], BF16, kind="ExternalInput")
    pw_d = nc.dram_tensor("pw", [D, D], BF16, kind="ExternalInput")
    cqk_d = nc.dram_tensor("cqk", [2 * D], F32, kind="ExternalInput")
    cvpb_d = nc.dram_tensor("cvpb", [2 * D], BF16, kind="ExternalInput")
    bmm_d = nc.dram_tensor("bmm", [NP2, NH * NP2], BF16, kind="ExternalInput")
    ident_d = nc.dram_tensor("ident", [NP2, NP2], BF16, kind="ExternalInput")
    y_d = nc.dram_tensor("y", [TOK_CORE, D], F32, kind="ExternalOutput")

    from contextlib import ExitStack
    with tile.TileContext(nc) as tc, ExitStack() as ctx:
        const = ctx.enter_context(tc.tile_pool(name="const", bufs=1))
        grp = ctx.enter_context(tc.tile_pool(name="grp", bufs=2))
        work = ctx.enter_context(tc.tile_pool(name="work", bufs=3))
        small = ctx.enter_context(tc.tile_pool(name="small", bufs=4))
        ps_mm = ctx.enter_context(tc.tile_pool(name="ps_mm", bufs=3, space="PSUM"))
        ps_s = ctx.enter_context(tc.tile_pool(name="ps_s", bufs=3, space="PSUM"))
        ps_o = ctx.enter_context(tc.tile_pool(name="ps_o", bufs=2, space="PSUM"))

        # ---- resident constants -------------------------------------------------
        qkw_sb = [const.tile([128, 2 * D], BF16, name=f"qkw{k}", tag=f"qkw{k}") for k in range(3)]
        vw_sb = [const.tile([128, D], BF16, name=f"vw{k}", tag=f"vw{k}") for k in range(3)]
        pw_sb = [const.tile([128, D], BF16, name=f"pw{k}", tag=f"pw{k}") for k in range(3)]
        for k in range(3):
            nc.sync.dma_start(out=qkw_sb[k][:], in_=qkw_d[128 * k:128 * (k + 1), :])
            nc.sync.dma_start(out=vw_sb[k][:], in_=vw_d[128 * k:128 * (k + 1), :])
            nc.sync.dma_start(out=pw_sb[k][:], in_=pw_d[128 * k:128 * (k + 1), :])
        cqk_sb = [const.tile([128, 1], F32, name=f"cqk{m}", tag=f"cqk{m}") for m in range(6)]
        for m in range(6):
            nc.sync.dma_start(out=cqk_sb[m][:], in_=cqk_d[128 * m:128 * (m + 1)])

        def row1(dram_ap):
            # DRAM vector -> [1, n] AP (single partition)
            return bass.AP(tensor=dram_ap.tensor, offset=dram_ap.offset,
                           ap=[[0, 1], *dram_ap.ap])

        cvrow = const.tile([128, D], BF16, name="cvrow", tag="cvrow")
        nc.sync.dma_start(out=cvrow[0:1, :], in_=row1(cvpb_d[0:D]))
        pbrow = const.tile([128, D], BF16, name="pbrow", tag="pbrow")
        nc.sync.dma_start(out=pbrow[0:1, :], in_=row1(cvpb_d[D:2 * D]))
        bmm_sb = const.tile([128, NH, NP2], BF16, name="bmm", tag="bmm")
        nc.sync.dma_start(out=bmm_sb[0:NP2, :, :], in_=bmm_d[:, :])
        ident_sb = const.tile([128, NP2], BF16, name="ident", tag="ident")
        nc.sync.dma_start(out=ident_sb[0:NP2, :], in_=ident_d[:, :])
        eps_sb = const.tile([128, 1], F32, name="eps", tag="eps")
        nc.vector.memset(eps_sb[:], EPS)
        ones1 = const.tile([128, NP2], BF16, name="ones1", tag="ones1")
        nc.gpsimd.memset(ones1[0:1, :], 1.0)

        NROT = 3
        av_rot = [const.tile([128, NH, HD + 1], BF16, name=f"av{i}", tag=f"av{i}") for i in range(NROT)]
        for t in av_rot:
            nc.gpsimd.memset(t[:, :, HD:HD + 1], 1.0)
        # rotating LN-output / attention-output tiles: pad rows 98-111
        # (read by the xbar transpose) are zeroed once
        xn_rot = [const.tile([128, D], BF16, name=f"xn{i}", tag=f"xn{i}") for i in range(NROT)]
        for t in xn_rot:
            # 32-aligned start; rows 96-97 are rewritten by every LN apply
            # before any transpose reads
            nc.gpsimd.memset(t[96:SLOT, :], 0.0)
        at_rot = [const.tile([128, D], BF16, name=f"atr{i}", tag=f"atr{i}") for i in range(NROT)]
        for t in at_rot:
            nc.gpsimd.memset(t[96:SLOT, :], 0.0)

        rep_ctx = tc.For_i(0, repeat, 1) if repeat > 1 else None
        if rep_ctx is not None:
            rep_ctx.__enter__()
        for g in range(N_GROUPS):
            R0 = g * TOK_G
            x_t = grp.tile([128, PAIRS_G, D], F32, name="x_t", tag="x_t")
            xT = grp.tile([128, 3, PAIRS_G, SLOT], BF16, name="xT", tag="xT")
            qk = [grp.tile([128, TOK_G], BF16, name=f"qk{m}", tag=f"qk{m}") for m in range(6)]
            yg = grp.tile([128, PAIRS_G, D], F32, name="yg", tag="yg")

            hp = PAIRS_G // 2
            for hf in range(2):
                src = x_d[R0 + hf * hp * NP2: R0 + (hf + 1) * hp * NP2, :]
                nc.sync.dma_start(
                    out=x_t[0:NP2, hf * hp:(hf + 1) * hp, :],
                    in_=src.rearrange("(p t) d -> t p d", t=NP2),
                )

            # ---- phase A: LN + xbar transpose ----------------------------------
            mv6 = small.tile([128, PAIRS_G, 6], F32, name="mv6", tag="mv6")
            mva = small.tile([128, PAIRS_G, 2], F32, name="mva", tag="mva")
            for p in range(PAIRS_G):
                nc.vector.bn_stats(out=mv6[0:NP2, p, :], in_=x_t[0:NP2, p, :])
                nc.vector.bn_aggr(out=mva[0:NP2, p, :], in_=mv6[0:NP2, p, :])
            lnv = small.tile([128, PAIRS_G], F32, name="lnv", tag="lnv")
            nc.scalar.activation(
                out=lnv[0:NP2, :], in_=mva[0:NP2, :, 1], func=AF.Ln,
                bias=eps_sb[0:NP2, :], scale=1.0,
            )
            rs = small.tile([128, PAIRS_G], F32, name="rs", tag="rs")
            nc.scalar.activation(
                out=rs[0:NP2, :], in_=lnv[0:NP2, :], func=AF.Exp, scale=-0.5,
            )
            for p in range(PAIRS_G):
                xn = xn_rot[p % NROT]
                nc.vector.tensor_scalar(
                    xn[0:NP2, :], x_t[0:NP2, p, :],
                    mva[0:NP2, p, 0:1], rs[0:NP2, p:p + 1],
                    op0=OP.subtract, op1=OP.mult,
                )
                for k in range(3):
                    nc.sync.dma_start(
                        out=xT[:, k, p, :],
                        in_=xn[0:SLOT, 128 * k:128 * (k + 1)],
                        transpose=True,
                    )

            # ---- phase B: q/k projections (feature-major) ----------------------
            for c in range(PAIRS_G // PCH):
                for m in range(6):
                    pq_full = ps_mm.tile([128, 512], F32, name="pmm", tag="pmm")
                    pq = pq_full[:, 0:PCH * NP2]
                    for k in range(3):
                        nc.tensor.matmul(
                            pq[:, :],
                            lhsT=qkw_sb[k][:, 128 * m:128 * (m + 1)],
                            rhs=xT[:, k, PCH * c:PCH * (c + 1), 0:NP2],
                            start=(k == 0), stop=(k == 2),
                        )
                    nc.scalar.activation(
                        out=qk[m][:, PCH * NP2 * c:PCH * NP2 * (c + 1)],
                        in_=pq[:, :], func=AF.Identity,
                        bias=cqk_sb[m][:], scale=1.0,
                    )

            # ---- phase C: per window pair --------------------------------------
            for p in range(PAIRS_G):
                # v projection (token-major, both windows stacked) + cv
                pv_full = ps_mm.tile([128, 512], F32, name="pmm", tag="pmm")
                pv = pv_full
                for k in range(3):
                    nc.tensor.matmul(
                        pv[0:NP2, 0:D], lhsT=xT[:, k, p, 0:NP2], rhs=vw_sb[k][:],
                        start=(k == 0), stop=False,
                    )
                nc.tensor.matmul(
                    pv[0:NP2, 0:D], lhsT=ones1[0:1, :], rhs=cvrow[0:1, :],
                    start=False, stop=True,
                )
                av = av_rot[p % NROT]
                nc.vector.tensor_scalar(
                    av[0:NP2, :, 0:HD],
                    pv[0:NP2, 0:D].rearrange("p (h d) -> p h d", d=HD),
                    0.0, None, op0=OP.add,
                )

                # scores S^T[k, q] (stacked pair); rel-pos bias + cross-window
                # mask ride a closing matmul (bmm^T via identity) per head
                p_t = work.tile([128, NH, NP2], BF16, name="pt", tag="pt")
                for quad in range(3):
                    ps_full = ps_s.tile([128, 512], F32, name="ps", tag="ps")
                    ps = ps_full[:, 0:4 * NP2].rearrange("p (j t) -> p j t", t=NP2)
                    qt = qk[quad]
                    kt = qk[3 + quad]
                    for j in range(4):
                        hb = 32 * j
                        h = 4 * quad + j
                        nc.tensor.matmul(
                            ps[0:NP2, j, :],
                            lhsT=kt[hb:hb + 32, NP2 * p:NP2 * (p + 1)],
                            rhs=qt[hb:hb + 32, NP2 * p:NP2 * (p + 1)],
                            start=True, stop=False,
                            tile_position=(hb, 0),
                        )
                        nc.tensor.matmul(
                            ps[0:NP2, j, :],
                            lhsT=bmm_sb[0:NP2, h, :],
                            rhs=ident_sb[0:NP2, :],
                            start=False, stop=True,
                        )
                    nc.scalar.activation(
                        out=p_t[0:NP2, 4 * quad:4 * (quad + 1), :],
                        in_=ps[0:NP2, :, :], func=AF.Exp,
                    )

                # P @ [V | 1]
                po_full = ps_o.tile([128, 512], F32, name="po", tag="po")
                po = po_full[:, 0:NH * (HD + 1)].rearrange("p (h d) -> p h d", d=HD + 1)
                for h in range(NH):
                    nc.tensor.matmul(
                        po[0:NP2, h, :], lhsT=p_t[0:NP2, h, :], rhs=av[0:NP2, h, :],
                        start=True, stop=True,
                    )
                rec = small.tile([128, NH], F32, name="rec", tag="rec")
                nc.vector.reciprocal(out=rec[0:NP2, :], in_=po[0:NP2, :, HD])
                at_sb = at_rot[p % NROT]
                rec_sl = rec[0:NP2, :]
                rec_b = bass.AP(
                    tensor=rec_sl.tensor, offset=rec_sl.offset,
                    ap=[*rec_sl.ap, [0, HD]],
                )
                nc.vector.tensor_tensor(
                    out=at_sb[0:NP2, :].rearrange("p (h d) -> p h d", d=HD),
                    in0=po[0:NP2, :, 0:HD], in1=rec_b, op=OP.mult,
                )

                # xbar-transpose attention out, proj (+pb), stage to yg
                atT = work.tile([128, 3, SLOT], BF16, name="atT", tag="atT")
                for k in range(3):
                    nc.sync.dma_start(
                        out=atT[:, k, :],
                        in_=at_sb[0:SLOT, 128 * k:128 * (k + 1)],
                        transpose=True,
                    )
                pp_full = ps_mm.tile([128, 512], F32, name="pmm", tag="pmm")
                pp = pp_full
                nc.tensor.matmul(
                    pp[0:NP2, 0:D], lhsT=ones1[0:1, :], rhs=pbrow[0:1, :],
                    start=True, stop=False,
                )
                for k in range(3):
                    nc.tensor.matmul(
                        pp[0:NP2, 0:D], lhsT=atT[:, k, 0:NP2], rhs=pw_sb[k][:],
                        start=False, stop=(k == 2),
                    )
                nc.scalar.activation(
                    out=yg[0:NP2, p, :], in_=pp[0:NP2, 0:D], func=AF.Identity,
                )

            for hf in range(2):
                dst = y_d[R0 + hf * hp * NP2: R0 + (hf + 1) * hp * NP2, :]
                nc.sync.dma_start(
                    out=dst.rearrange("(p t) d -> t p d", t=NP2),
                    in_=yg[0:NP2, hf * hp:(hf + 1) * hp, :],
                )
        if rep_ctx is not None:
            rep_ctx.__exit__(None, None, None)

    nc.compile()
    return nc


_NC_CACHE = {}


def _get_program():
    if "nc" not in _NC_CACHE:
        _NC_CACHE["nc"] = build_program()
    return _NC_CACHE["nc"]


def _window_order(xf):
    # [BT, H, W, D] -> [BT*nW*N, D] in window-raster order
    BT = xf.shape[0]
    x6 = xf.reshape(BT, H // WSZ, WSZ, W // WSZ, WSZ, D)
    return np.ascontiguousarray(x6.transpose(0, 1, 3, 2, 4, 5)).reshape(-1, D)


def _window_unorder(yw):
    BT = B * T
    y6 = yw.reshape(BT, H // WSZ, W // WSZ, WSZ, WSZ, D)
    return np.ascontiguousarray(y6.transpose(0, 1, 3, 2, 4, 5)).reshape(BT, H, W, D)


def prepare_inputs(x, ln_g, ln_b, qkv_w, qkv_b, proj_w, proj_b, rel_bias_table):
    x = np.asarray(x, np.float32)
    ln_g = np.asarray(ln_g, np.float32)
    ln_b = np.asarray(ln_b, np.float32)
    qkv_w = np.asarray(qkv_w, np.float32)
    qkv_b = np.asarray(qkv_b, np.float32)
    proj_w = np.asarray(proj_w, np.float32)
    proj_b = np.asarray(proj_b, np.float32)
    rel_bias_table = np.asarray(rel_bias_table, np.float32)

    scale = HD ** -0.5
    wq = qkv_w[:, :D] * ln_g[:, None] * scale
    wk = qkv_w[:, D:2 * D] * ln_g[:, None]
    wv = qkv_w[:, 2 * D:] * ln_g[:, None]
    cq = (ln_b @ qkv_w[:, :D] + qkv_b[:D]) * scale
    ck = ln_b @ qkv_w[:, D:2 * D] + qkv_b[D:2 * D]
    cv = ln_b @ qkv_w[:, 2 * D:] + qkv_b[2 * D:]

    qkw = np.concatenate([wq, wk], axis=1).astype(ml_dtypes.bfloat16)
    cqk = np.concatenate([cq, ck]).astype(np.float32)
    cvpb = np.concatenate([cv, proj_b]).astype(ml_dtypes.bfloat16)

    idx = _rel_index(WSZ)
    bias = rel_bias_table[idx.reshape(-1)].reshape(N, N, NH)  # [q, k, h]
    # bmm[q, h*98+k] = bias[h][q, k] on same-window blocks, -1e30 across
    # (exp of masked scores is then exactly 0 -> correct windowed softmax)
    bmm = np.full((NP2, NH, NP2), -1e30, np.float32)
    for h in range(NH):
        blk = bias[:, :, h]  # [q, k]
        bmm[0:N, h, 0:N] = blk
        bmm[N:NP2, h, N:NP2] = blk
    bmm = bmm.reshape(NP2, NH * NP2).astype(ml_dtypes.bfloat16)

    ident = np.eye(NP2, dtype=np.float32).astype(ml_dtypes.bfloat16)

    xw = _window_order(x.reshape(B * T, H, W, D))

    common = {
        "qkw": qkw,
        "vw": wv.astype(ml_dtypes.bfloat16),
        "pw": proj_w.astype(ml_dtypes.bfloat16),
        "cqk": cqk,
        "cvpb": cvpb,
        "bmm": bmm,
        "ident": ident,
    }
    in_maps = []
    for c in range(N_CORES):
        m = dict(common)
        m["x"] = np.ascontiguousarray(xw[TOK_CORE * c:TOK_CORE * (c + 1)])
        in_maps.append(m)
    return in_maps


def kernel(x, ln_g, ln_b, qkv_w, qkv_b, proj_w, proj_b, rel_bias_table):
    nc = _get_program()
    in_maps = prepare_inputs(x, ln_g, ln_b, qkv_w, qkv_b, proj_w, proj_b, rel_bias_table)
    res = run_bass_kernel_spmd(nc, in_maps, core_ids=list(range(N_CORES)))
    yw = np.concatenate([res.results[c]["y"] for c in range(N_CORES)], axis=0)
    out = _window_unorder(yw).reshape(B, T, H, W, D)
    return out.astype(np.float32)


# revision 25
# speedup vs baseline: 95902.0885x; 4.2254x over previous
"""Swin-style windowed local self-attention (LN -> QKV -> 7x7 window MHA
with relative position bias -> proj) on 8 Trainium2 NeuronCores.

Sharding: pure data parallel over B*T (24 images -> 3 per core).

v3 design (per core: 9408 tokens = 96 window pairs, 8 groups of 12):
  - window PAIRS are stacked: 98 tokens (2 windows) live on partitions
    0-97 of every working tile; relative-position bias AND the
    cross-window mask (-1e30) are accumulated by a closing matmul
    (bmm^T via identity) per head -- this also works around a HW issue
    with >2 concurrent single-shot tile_position matmuls per PSUM bank.
  - x^T / attention-out transposes use the DMA xbar (112-wide slots for
    16-alignment), keeping the tensor engine free for matmuls.
  - LayerNorm rsqrt is computed as exp(-0.5*ln(var+eps)) so the scalar
    engine only uses the natural_log_exp_and_others activation table
    (ln/exp/identity): no activation-table reloads.
  - x is loaded with 2 big strided DMAs per group; y is staged in a
    group SBUF tile and stored with 2 big DMAs per group.
  - softmax normalization: V is augmented with a ones column, so P@V
    also yields row sums; reciprocal + per-partition broadcast multiply
    normalizes at PSUM evacuation.
  - LN bias/scale, qkv bias, v bias and proj bias are folded on the
    host; cv/pb ride the PE as rank-1 matmuls (ones-row lhsT).
"""

import sys

if "/opt/trn_rl_repo" not in sys.path:
    sys.path.insert(0, "/opt/trn_rl_repo")

import numpy as np
import ml_dtypes

import concourse.bacc as bacc
import concourse.bass as bass
import concourse.tile as tile
import concourse.mybir as mybir
from concourse.bass_utils import run_bass_kernel_spmd

F32 = mybir.dt.float32
BF16 = mybir.dt.bfloat16
AF = mybir.ActivationFunctionType
OP = mybir.AluOpType

N_CORES = 8
B, T, H, W, D = 4, 6, 56, 56, 384
WSZ = 7
NH = 12
HD = D // NH            # 32
N = WSZ * WSZ           # 49 tokens / window
IMGS_CORE = (B * T) // N_CORES     # 3
TOK_CORE = IMGS_CORE * H * W       # 9408
NPAIR = TOK_CORE // (2 * N)        # 96 window pairs / core
NP2 = 2 * N                        # 98 tokens / pair
SLOT = 112                         # 16-aligned token slot for xbar transposes
EPS = 1e-5

N_GROUPS = 12
PAIRS_G = NPAIR // N_GROUPS        # 8
TOK_G = PAIRS_G * NP2              # 1176
PCH = 4                            # pairs per q/k projection chunk


def _rel_index(w):
    coords = np.stack(np.meshgrid(np.arange(w), np.arange(w), indexing="ij")).reshape(2, -1)
    rel = coords[:, :, None] - coords[:, None, :]
    return (rel[0] + w - 1) * (2 * w - 1) + (rel[1] + w - 1)


def build_program(repeat=1, abl=''):
    nc = bacc.Bacc("TRN2", target_bir_lowering=False, debug=False, num_devices=N_CORES)

    x_d = nc.dram_tensor("x", [TOK_CORE, D], F32, kind="ExternalInput")
    qkw_d = nc.dram_tensor("qkw", [D, 2 * D], BF16, kind="ExternalInput")
    vw_d = nc.dram_tensor("vw", [D, D], BF16, kind="ExternalInput")
    pw_d = nc.dram_tensor("pw", [D, D], BF16, kind="ExternalInput")
    cqk_d = nc.dram_tensor("cqk", [2 * D], F32, kind="ExternalInput")
    cvpb_d = nc.dram_tensor("cvpb", [2 * D], BF16, kind="ExternalInput")
    bmm_d = nc.dram_tensor("bmm", [128, NH * NP2], BF16, kind="ExternalInput")
    ident_d = nc.dram_tensor("ident", [128, NP2], BF16, kind="ExternalInput")
    y_d = nc.dram_tensor("y", [TOK_CORE, D], F32, kind="ExternalOutput")

    from contextlib import ExitStack
    with tile.TileContext(nc) as tc, ExitStack() as ctx:
        const = ctx.enter_context(tc.tile_pool(name="const", bufs=1))
        grp = ctx.enter_context(tc.tile_pool(name="grp", bufs=2))
        work = ctx.enter_context(tc.tile_pool(name="work", bufs=3))
        small = ctx.enter_context(tc.tile_pool(name="small", bufs=4))
        ps_mm = ctx.enter_context(tc.tile_pool(name="ps_mm", bufs=3, space="PSUM"))
        ps_s = ctx.enter_context(tc.tile_pool(name="ps_s", bufs=3, space="PSUM"))
        ps_o = ctx.enter_context(tc.tile_pool(name="ps_o", bufs=2, space="PSUM"))

        # ---- resident constants -------------------------------------------------
        qkw_sb = [const.tile([128, 2 * D], BF16, name=f"qkw{k}", tag=f"qkw{k}") for k in range(3)]
        vw_sb = [const.tile([128, D], BF16, name=f"vw{k}", tag=f"vw{k}") for k in range(3)]
        pw_sb = [const.tile([128, D], BF16, name=f"pw{k}", tag=f"pw{k}") for k in range(3)]
        for k in range(3):
            nc.sync.dma_start(out=qkw_sb[k][:], in_=qkw_d[128 * k:128 * (k + 1), :])
            nc.sync.dma_start(out=vw_sb[k][:], in_=vw_d[128 * k:128 * (k + 1), :])
            nc.sync.dma_start(out=pw_sb[k][:], in_=pw_d[128 * k:128 * (k + 1), :])
        cqk_sb = [const.tile([128, 1], F32, name=f"cqk{m}", tag=f"cqk{m}") for m in range(6)]
        for m in range(6):
            nc.sync.dma_start(out=cqk_sb[m][:], in_=cqk_d[128 * m:128 * (m + 1)])

        def row1(dram_ap):
            # DRAM vector -> [1, n] AP (single partition)
            return bass.AP(tensor=dram_ap.tensor, offset=dram_ap.offset,
                           ap=[[0, 1], *dram_ap.ap])

        def bcast128(dram_ap):
            return bass.AP(tensor=dram_ap.tensor, offset=dram_ap.offset,
                           ap=[[0, 128], *dram_ap.ap])

        cv_sb = const.tile([128, D], BF16, name="cv", tag="cv")
        nc.sync.dma_start(out=cv_sb[:, :], in_=bcast128(cvpb_d[0:D]))
        pbrow = const.tile([128, D], BF16, name="pbrow", tag="pbrow")
        nc.sync.dma_start(out=pbrow[0:1, :], in_=row1(cvpb_d[D:2 * D]))
        bmm_sb = const.tile([128, NH, NP2], BF16, name="bmm", tag="bmm")
        nc.sync.dma_start(out=bmm_sb[:, :, :], in_=bmm_d[:, :])
        ident_sb = const.tile([128, NP2], BF16, name="ident", tag="ident")
        nc.sync.dma_start(out=ident_sb[:, :], in_=ident_d[:, :])
        eps_sb = const.tile([128, 1], F32, name="eps", tag="eps")
        nc.vector.memset(eps_sb[:], EPS)
        ones1 = const.tile([128, NP2], BF16, name="ones1", tag="ones1")
        nc.gpsimd.memset(ones1[0:1, :], 1.0)

        NROT = PAIRS_G
        av_rot = [const.tile([128, NH, HD + 1], BF16, name=f"av{i}", tag=f"av{i}") for i in range(NROT)]
        for t in av_rot:
            nc.gpsimd.memset(t[:, :, HD:HD + 1], 1.0)
            # rows 98-127 are contracted against p_t zero-pads: 0 * NaN = NaN,
            # so they must be finite
            nc.gpsimd.memset(t[96:128, :, 0:HD], 0.0)
        # rotating LN-output / attention-output tiles: pad rows 98-111
        # (read by the xbar transpose) are zeroed once
        xn_rot = [const.tile([128, D], BF16, name=f"xn{i}", tag=f"xn{i}") for i in range(NROT)]
        for t in xn_rot:
            # 32-aligned start; rows 96-97 are rewritten by every LN apply
            # before any transpose reads
            nc.gpsimd.memset(t[96:SLOT, :], 0.0)
        at_rot = [const.tile([128, D], BF16, name=f"atr{i}", tag=f"atr{i}") for i in range(NROT)]
        for t in at_rot:
            nc.gpsimd.memset(t[96:SLOT, :], 0.0)
        # p_t pad rows 98-127 zeroed once -> PV can contract over full 128
        # partitions (FWL); rows 96-97 are rewritten by every exp
        pt_rot = [const.tile([128, NH, NP2], BF16, name=f"pt{i}", tag=f"pt{i}") for i in range(NROT)]
        for t in pt_rot:
            nc.gpsimd.memset(t[96:128, :, :], 0.0)
        atT_rot = [const.tile([128, 3, SLOT], BF16, name=f"atT{i}", tag=f"atT{i}") for i in range(NROT)]

        rep_ctx = tc.For_i(0, repeat, 1) if repeat > 1 else None
        if rep_ctx is not None:
            rep_ctx.__enter__()
        for g in range(N_GROUPS):
            R0 = g * TOK_G
            x_t = grp.tile([128, PAIRS_G, D], F32, name="x_t", tag="x_t")
            xT = grp.tile([128, PAIRS_G, 3, SLOT], BF16, name="xT", tag="xT")
            qk = [grp.tile([128, TOK_G], BF16, name=f"qk{m}", tag=f"qk{m}") for m in range(6)]
            yg = grp.tile([128, PAIRS_G, D], F32, name="yg", tag="yg")

            hp = PAIRS_G // 2
            for hf in range(2):
                src = x_d[R0 + hf * hp * NP2: R0 + (hf + 1) * hp * NP2, :]
                nc.sync.dma_start(
                    out=x_t[0:NP2, hf * hp:(hf + 1) * hp, :],
                    in_=src.rearrange("(p t) d -> t p d", t=NP2),
                )

            # ---- phase A: LN + xbar transpose ----------------------------------
            mv6 = small.tile([128, PAIRS_G, 6], F32, name="mv6", tag="mv6")
            mva = small.tile([128, PAIRS_G, 2], F32, name="mva", tag="mva")
            for p in range(PAIRS_G if abl != 'noln' else 0):
                nc.vector.bn_stats(out=mv6[0:NP2, p, :], in_=x_t[0:NP2, p, :])
                nc.vector.bn_aggr(out=mva[0:NP2, p, :], in_=mv6[0:NP2, p, :])
            lnv = small.tile([128, PAIRS_G], F32, name="lnv", tag="lnv")
            nc.scalar.activation(
                out=lnv[0:NP2, :], in_=mva[0:NP2, :, 1], func=AF.Ln,
                bias=eps_sb[0:NP2, :], scale=1.0,
            )
            rs = small.tile([128, PAIRS_G], F32, name="rs", tag="rs")
            nc.scalar.activation(
                out=rs[0:NP2, :], in_=lnv[0:NP2, :], func=AF.Exp, scale=-0.5,
            )
            nm = small.tile([128, PAIRS_G], F32, name="nm", tag="nm")
            nc.vector.tensor_tensor(
                out=nm[0:NP2, :], in0=mva[0:NP2, :, 0], in1=rs[0:NP2, :], op=OP.mult,
            )
            nc.vector.tensor_scalar(
                nm[0:NP2, :], nm[0:NP2, :], -1.0, None, op0=OP.mult,
            )
            for p in range(PAIRS_G):
                xn = xn_rot[0] if abl == 'noln' else xn_rot[p % NROT]
                if abl == 'noln':
                    if p == 0 and g == 0:
                        nc.vector.memset(xn[0:NP2, :], 0.5)
                    for _ in range(1):
                        nc.sync.dma_start(
                            out=xT[:, p, :, :], in_=xn[0:SLOT, :], transpose=True,
                        )
                    continue
                nc.scalar.activation(
                    out=xn[0:NP2, :], in_=x_t[0:NP2, p, :], func=AF.Identity,
                    bias=nm[0:NP2, p:p + 1], scale=rs[0:NP2, p:p + 1],
                )
                if abl != 'noxbar':
                    nc.sync.dma_start(
                        out=xT[:, p, :, :], in_=xn[0:SLOT, :], transpose=True,
                    )

            # ---- phase B: q/k projections (feature-major) ----------------------
            for c in range(PAIRS_G // PCH):
                for m in range(6):
                    pq_full = ps_mm.tile([128, 512], F32, name="pmm", tag="pmm")
                    pq = pq_full[:, 0:PCH * NP2]
                    for k in range(3):
                        nc.tensor.matmul(
                            pq[:, :],
                            lhsT=qkw_sb[k][:, 128 * m:128 * (m + 1)],
                            rhs=xT[:, PCH * c:PCH * (c + 1), k, 0:NP2],
                            start=(k == 0), stop=(k == 2),
                        )
                    nc.vector.tensor_scalar(
                        qk[m][:, PCH * NP2 * c:PCH * NP2 * (c + 1)],
                        pq[:, :], cqk_sb[m][:], None, op0=OP.add,
                    )

            # ---- phase C, stage 1: v projection + scores + exp ------------------
            for p in range(PAIRS_G):
                pv_full = ps_mm.tile([128, 512], F32, name="pmm", tag="pmm")
                pv = pv_full
                for k in range(3):
                    nc.tensor.matmul(
                        pv[0:NP2, 0:D], lhsT=xT[:, p, k, 0:NP2], rhs=vw_sb[k][:],
                        start=(k == 0), stop=(k == 2),
                    )
                av = av_rot[p]
                nc.vector.tensor_tensor(
                    out=av[0:NP2, :, 0:HD],
                    in0=pv[0:NP2, 0:D].rearrange("p (h d) -> p h d", d=HD),
                    in1=cv_sb[0:NP2, :].rearrange("p (h d) -> p h d", d=HD),
                    op=OP.add,
                )
            for p in range(PAIRS_G):
                p_t = pt_rot[p]
                for quad in range(2 if abl == 'heads6' else 3):
                    ps_full = ps_s.tile([128, 512], F32, name="ps", tag="ps")
                    ps = ps_full[:, 0:4 * NP2].rearrange("p (j t) -> p j t", t=NP2)
                    qt = qk[quad]
                    kt = qk[3 + quad]
                    for j in range(4):
                        hb = 32 * j
                        h = 4 * quad + j
                        nc.tensor.matmul(
                            ps[0:NP2, j, :],
                            lhsT=kt[hb:hb + 32, NP2 * p:NP2 * (p + 1)],
                            rhs=qt[hb:hb + 32, NP2 * p:NP2 * (p + 1)],
                            start=True, stop=False,
                            tile_position=(hb, 0),
                        )
                        nc.tensor.matmul(
                            ps[0:NP2, j, :],
                            lhsT=bmm_sb[:, h, :],
                            rhs=ident_sb[:, :],
                            start=False, stop=True,
                        )
                    nc.scalar.activation(
                        out=p_t[0:NP2, 4 * quad:4 * (quad + 1), :],
                        in_=ps[0:NP2, :, :], func=AF.Exp,
                    )

            # ---- phase C, stage 2: P @ [V|1], normalize, transpose back ---------
            for p in range(PAIRS_G):
                p_t = pt_rot[p]
                av = av_rot[p]
                po_full = ps_o.tile([128, 512], F32, name="po", tag="po")
                po = po_full[:, 0:NH * (HD + 1)].rearrange("p (h d) -> p h d", d=HD + 1)
                for h in range(8 if abl == 'heads6' else NH):
                    nc.tensor.matmul(
                        po[0:NP2, h, :], lhsT=p_t[:, h, :], rhs=av[:, h, :],
                        start=True, stop=True,
                    )
                rec = small.tile([128, NH], F32, name="rec", tag="rec")
                nc.vector.reciprocal(out=rec[0:NP2, :], in_=po[0:NP2, :, HD])
                at_sb = at_rot[p % NROT]
                rec_sl = rec[0:NP2, :]
                rec_b = bass.AP(
                    tensor=rec_sl.tensor, offset=rec_sl.offset,
                    ap=[*rec_sl.ap, [0, HD]],
                )
                nc.vector.tensor_tensor(
                    out=at_sb[0:NP2, :].rearrange("p (h d) -> p h d", d=HD),
                    in0=po[0:NP2, :, 0:HD], in1=rec_b, op=OP.mult,
                )
                atT = atT_rot[p]
                if abl == 'noat':
                    nc.gpsimd.memset(atT[:, :, :], 0.5)
                if abl != 'noat':
                    nc.sync.dma_start(
                        out=atT[:, :, :], in_=at_sb[0:SLOT, :], transpose=True,
                    )

            # ---- phase C, stage 3: proj (+pb), stage to yg ----------------------
            for p in range(PAIRS_G):
                atT = atT_rot[p]
                pp_full = ps_mm.tile([128, 512], F32, name="pmm", tag="pmm")
                pp = pp_full
                nc.tensor.matmul(
                    pp[0:NP2, 0:D], lhsT=ones1[0:1, :], rhs=pbrow[0:1, :],
                    start=True, stop=False,
                )
                for k in range(3):
                    nc.tensor.matmul(
                        pp[0:NP2, 0:D], lhsT=atT[:, k, 0:NP2], rhs=pw_sb[k][:],
                        start=False, stop=(k == 2),
                    )
                nc.scalar.activation(
                    out=yg[0:NP2, p, :], in_=pp[0:NP2, 0:D], func=AF.Identity,
                )

            for hf in range(2):
                dst = y_d[R0 + hf * hp * NP2: R0 + (hf + 1) * hp * NP2, :]
                nc.sync.dma_start(
                    out=dst.rearrange("(p t) d -> t p d", t=NP2),
                    in_=yg[0:NP2, hf * hp:(hf + 1) * hp, :],
                )
        if rep_ctx is not None:
            rep_ctx.__exit__(None, None, None)

    nc.compile()
    return nc


_NC_CACHE = {}


def _get_program():
    if "nc" not in _NC_CACHE:
        _NC_CACHE["nc"] = build_program()
    return _NC_CACHE["nc"]


def _window_order(xf):
    # [BT, H, W, D] -> [BT*nW*N, D] in window-raster order
    BT = xf.shape[0]
    x6 = xf.reshape(BT, H // WSZ, WSZ, W // WSZ, WSZ, D)
    return np.ascontiguousarray(x6.transpose(0, 1, 3, 2, 4, 5)).reshape(-1, D)


def _window_unorder(yw):
    BT = B * T
    y6 = yw.reshape(BT, H // WSZ, W // WSZ, WSZ, WSZ, D)
    return np.ascontiguousarray(y6.transpose(0, 1, 3, 2, 4, 5)).reshape(BT, H, W, D)


def prepare_inputs(x, ln_g, ln_b, qkv_w, qkv_b, proj_w, proj_b, rel_bias_table):
    x = np.asarray(x, np.float32)
    ln_g = np.asarray(ln_g, np.float32)
    ln_b = np.asarray(ln_b, np.float32)
    qkv_w = np.asarray(qkv_w, np.float32)
    qkv_b = np.asarray(qkv_b, np.float32)
    proj_w = np.asarray(proj_w, np.float32)
    proj_b = np.asarray(proj_b, np.float32)
    rel_bias_table = np.asarray(rel_bias_table, np.float32)

    scale = HD ** -0.5
    wq = qkv_w[:, :D] * ln_g[:, None] * scale
    wk = qkv_w[:, D:2 * D] * ln_g[:, None]
    wv = qkv_w[:, 2 * D:] * ln_g[:, None]
    cq = (ln_b @ qkv_w[:, :D] + qkv_b[:D]) * scale
    ck = ln_b @ qkv_w[:, D:2 * D] + qkv_b[D:2 * D]
    cv = ln_b @ qkv_w[:, 2 * D:] + qkv_b[2 * D:]

    qkw = np.concatenate([wq, wk], axis=1).astype(ml_dtypes.bfloat16)
    cqk = np.concatenate([cq, ck]).astype(np.float32)
    cvpb = np.concatenate([cv, proj_b]).astype(ml_dtypes.bfloat16)

    idx = _rel_index(WSZ)
    bias = rel_bias_table[idx.reshape(-1)].reshape(N, N, NH)  # [q, k, h]
    # bmm[q, h*98+k] = bias[h][q, k] on same-window blocks, -1e30 across
    # (exp of masked scores is then exactly 0 -> correct windowed softmax)
    bmm = np.full((128, NH, NP2), -1e30, np.float32)
    bmm[NP2:] = 0.0  # zero contract-pad rows (FWL-friendly 128-row weights)
    for h in range(NH):
        blk = bias[:, :, h]  # [q, k]
        bmm[0:N, h, 0:N] = blk
        bmm[N:NP2, h, N:NP2] = blk
    bmm = bmm.reshape(128, NH * NP2).astype(ml_dtypes.bfloat16)

    ident = np.zeros((128, NP2), np.float32)
    ident[0:NP2, 0:NP2] = np.eye(NP2)
    ident = ident.astype(ml_dtypes.bfloat16)

    xw = _window_order(x.reshape(B * T, H, W, D))

    common = {
        "qkw": qkw,
        "vw": wv.astype(ml_dtypes.bfloat16),
        "pw": proj_w.astype(ml_dtypes.bfloat16),
        "cqk": cqk,
        "cvpb": cvpb,
        "bmm": bmm,
        "ident": ident,
    }
    in_maps = []
    for c in range(N_CORES):
        m = dict(common)
        m["x"] = np.ascontiguousarray(xw[TOK_CORE * c:TOK_CORE * (c + 1)])
        in_maps.append(m)
    return in_maps


def kernel(x, ln_g, ln_b, qkv_w, qkv_b, proj_w, proj_b, rel_bias_table):
    nc = _get_program()
    in_maps = prepare_inputs(x, ln_g, ln_b, qkv_w, qkv_b, proj_w, proj_b, rel_bias_table)
    res = run_bass_kernel_spmd(nc, in_maps, core_ids=list(range(N_CORES)))
    yw = np.concatenate([res.results[c]["y"] for c in range(N_CORES)], axis=0)
    out = _window_unorder(yw).reshape(B, T, H, W, D)
    return out.astype(np.float32)
